# revision 1
# baseline (speedup 1.0000x reference)
"""RandomErasing kernel for Trainium2 (Bass/Tile), 8-core data parallel.

Reference semantics (per sample b):
    out[h,w,c] = noise[h,w,c] if (ch-hh <= h < ch+hh) and (cw-hw <= w < cw+hw)
                 else images[h,w,c]

Sharding: pure data parallel — 8 samples per NeuronCore (batch 64 / 8 cores).

Per-core layout: the 8-sample slab [8,224,224,3] f32 = 1,204,224 elements is
viewed as SBUF-shaped [128 partitions, 9408]: each partition holds exactly 14
consecutive image rows (9408 = 14*672, 672 = W*C) and each sample spans
exactly 16 partitions (224 rows / 14). The erase mask is rank-1 per sample
(row-flag x col-flag), built on-chip from iotas + per-partition tensor_scalar
compares against the runtime rectangle bounds, then applied with
copy_predicated over the noise tile into the image tile.
"""

import os

import numpy as np

B, H, W, C = 64, 224, 224, 3
M = 8                 # cores
PB = B // M           # samples per core = 8
P = 128               # SBUF partitions
WC = W * C            # 672 elements per image row
RPP = (PB * H) // P   # image rows per partition = 14
FREE = RPP * WC       # 9408 f32 per partition
SPP = P // PB         # partitions per sample = 16
# Per-chunk row counts (sum to RPP=14). Four chunks with a small tail chunk
# was the TimelineSim optimum: enough pipeline depth to keep the DMA engines
# saturated, and a short final compute+store tail.
CHUNKS = [4, 4, 4, 2]
assert sum(CHUNKS) == RPP, CHUNKS
DBUFS = 4
MBUFS = 3
# Stores go out on the ACT HWDGE ring, loads on the SP ring: separate FIFOs,
# so a store waiting on compute can never head-of-line-block a load.
STORE_ENG = "scalar"   # sync | scalar
MASK_ENG = "vector"    # vector | gpsimd

_cache = {}

LAST_RESULTS = None   # BassKernelResults of the most recent run (for profiling)


def _build_nc():
    import concourse.bacc as bacc
    import concourse.mybir as mybir
    import concourse.tile as tile

    f32 = mybir.dt.float32
    i32 = mybir.dt.int32
    Op = mybir.AluOpType

    # Bacc (not raw Bass): its compile() pass splits multi-wait sync into
    # event semaphores — TRN2 engine instructions take at most one wait.
    nc = bacc.Bacc("TRN2", target_bir_lowering=False, debug=False)
    # img and noise interleaved per partition ([p, 0, :]=images, [p, 1, :]=
    # noise) so one chunk needs one DMA — keeps the sync-wait count of the
    # consuming copy_predicated within the ISA slot budget.
    data = nc.dram_tensor("data", [P, 2, FREE], f32, kind="ExternalInput")
    # meta columns (one row per sample): 0=center_h 1=center_w 2=half_h
    # 3=half_w 4=base_row(=224*sample). Loaded as 8 tiny descriptors and
    # broadcast to all 128 partitions with a K=8 indicator matmul on the idle
    # PE, so no [128 x 20B] small-descriptor DMA occupies the saturated SDMA
    # engines.
    meta = nc.dram_tensor("meta", [PB, 5], i32, kind="ExternalInput")
    out = nc.dram_tensor("out", [P, FREE], f32, kind="ExternalOutput")

    with tile.TileContext(nc) as tc:
        with (
            tc.tile_pool(name="cpool", bufs=1) as cpool,
            tc.tile_pool(name="dpool", bufs=DBUFS) as dpool,
            tc.tile_pool(name="mpool", bufs=MBUFS) as mpool,
            tc.tile_pool(name="ppool", bufs=1, space="PSUM") as ppool,
        ):
            # meta rides the SWDGE (gpsimd) ring so it never queues ahead of
            # the first big image load on the SP HWDGE FIFO.
            meta_i8 = cpool.tile([PB, 5], i32, tag="meta_i8")
            nc.gpsimd.dma_start(out=meta_i8[:], in_=meta[:])
            meta_f8 = cpool.tile([PB, 5], f32, tag="meta_f8")
            nc.vector.tensor_copy(meta_f8[:], meta_i8[:])
            # E[b, p] = 1.0 iff partition p belongs to sample b (p//16 == b)
            e_iota = cpool.tile([PB, P], i32, tag="e_iota")
            nc.gpsimd.iota(e_iota[:], pattern=[[1, P]], base=0,
                           channel_multiplier=-SPP)
            e_ge = cpool.tile([PB, P], f32, tag="e_ge")
            nc.vector.tensor_scalar(e_ge[:], e_iota[:], 0.0, None, Op.is_ge)
            e_mat = cpool.tile([PB, P], f32, tag="e_mat")
            nc.vector.scalar_tensor_tensor(e_mat[:], e_iota[:], float(SPP),
                                           e_ge[:], Op.is_lt, Op.mult)
            meta_ps = ppool.tile([P, 5], f32, tag="meta_ps")
            nc.tensor.matmul(meta_ps[:], e_mat[:], meta_f8[:],
                             start=True, stop=True)
            meta_f = cpool.tile([P, 5], f32, tag="meta_f")
            nc.vector.tensor_copy(meta_f[:], meta_ps[:])
            ch, cw, hh, hw, base = (meta_f[:, j : j + 1] for j in range(5))

            # Rectangle bounds, one per partition (f32, exact for these ranges).
            # Rows are compared in global coordinates g = 14*p + r; adding
            # base=224*s to the per-sample bounds makes clamping unnecessary
            # because g never leaves its own sample's row range.
            bnd = cpool.tile([P, 4], f32, tag="bnd")
            r0, r1, c0, c1 = (bnd[:, j : j + 1] for j in range(4))
            nc.vector.tensor_scalar(r0, ch, hh, base, Op.subtract, Op.add)
            nc.vector.tensor_scalar(r1, ch, hh, base, Op.add, Op.add)
            nc.vector.tensor_scalar(c0, cw, hw, 3.0, Op.subtract, Op.mult)
            nc.vector.tensor_scalar(c1, cw, hw, 3.0, Op.add, Op.mult)

            # iota runs on GpSimd; bounce through a DVE tensor_copy so the
            # downstream tensor_scalar ops depend only on DVE-produced tiles
            # (the TS ISA slot fits a single sync-wait command).
            iota_g0 = cpool.tile([P, RPP], i32, tag="iota_g0")
            nc.gpsimd.iota(iota_g0[:], pattern=[[1, RPP]], base=0,
                           channel_multiplier=RPP)
            iota_e0 = cpool.tile([P, WC], i32, tag="iota_e0")
            nc.gpsimd.iota(iota_e0[:], pattern=[[1, WC]], base=0,
                           channel_multiplier=0)
            iota_g = cpool.tile([P, RPP], i32, tag="iota_g")
            nc.vector.tensor_copy(iota_g[:], iota_g0[:])
            iota_e = cpool.tile([P, WC], i32, tag="iota_e")
            nc.vector.tensor_copy(iota_e[:], iota_e0[:])

            # rowm[p, r] = 1.0 if global row 14p+r is inside the sample's
            # erase-row range; colm[p, e] = 1.0 if flattened column e (=3w+c)
            # is inside the erase-col range.
            rowm_ge = cpool.tile([P, RPP], f32, tag="rowm_ge")
            rowm = cpool.tile([P, RPP], f32, tag="rowm")
            nc.vector.tensor_scalar(rowm_ge[:], iota_g[:], r0, None, Op.is_ge)
            nc.vector.scalar_tensor_tensor(rowm[:], iota_g[:], r1, rowm_ge[:],
                                           Op.is_lt, Op.mult)
            colm_ge = cpool.tile([P, WC], f32, tag="colm_ge")
            colm = cpool.tile([P, WC], f32, tag="colm")
            nc.vector.tensor_scalar(colm_ge[:], iota_e[:], c0, None, Op.is_ge)
            nc.vector.scalar_tensor_tensor(colm[:], iota_e[:], c1, colm_ge[:],
                                           Op.is_lt, Op.mult)

            row0 = 0
            maxch = max(CHUNKS) * WC
            for rows in CHUNKS:
                chunk = rows * WC
                sl = slice(row0 * WC, row0 * WC + chunk)
                tdata = dpool.tile([P, 2 * maxch], f32, tag="tdata")
                # copy_predicated requires an integer mask dtype
                tmask = mpool.tile([P, maxch], mybir.dt.uint8, tag="tmask")
                nc.sync.dma_start(out=tdata[:, : 2 * chunk], in_=data[:, :, sl])
                timg = tdata[:, :chunk]
                tnoi = tdata[:, chunk : 2 * chunk]
                mask_eng = getattr(nc, MASK_ENG)
                for r in range(rows):
                    g = row0 + r
                    mask_eng.tensor_scalar(
                        tmask[:, r * WC : (r + 1) * WC], colm[:],
                        rowm[:, g : g + 1], None, Op.mult)
                nc.vector.copy_predicated(timg, tmask[:, :chunk], tnoi)
                getattr(nc, STORE_ENG).dma_start(out=out[:, sl], in_=timg)
                row0 += rows

    nc.compile()
    return nc


def _get_nc():
    if "nc" not in _cache:
        _cache["nc"] = _build_nc()
    return _cache["nc"]


def _make_in_maps(images, noise, center_h, center_w, half_h, half_w):
    images = np.ascontiguousarray(np.asarray(images, dtype=np.float32))
    noise = np.ascontiguousarray(np.asarray(noise, dtype=np.float32))
    center_h = np.asarray(center_h, dtype=np.int32)
    center_w = np.asarray(center_w, dtype=np.int32)
    half_h = np.asarray(half_h, dtype=np.int32)
    half_w = np.asarray(half_w, dtype=np.int32)

    base = np.arange(PB, dtype=np.int32) * H
    in_maps = []
    for i in range(M):
        sl = slice(i * PB, (i + 1) * PB)
        meta = np.stack(
            [center_h[sl], center_w[sl], half_h[sl], half_w[sl], base],
            axis=1).astype(np.int32)
        in_maps.append({
            "data": np.ascontiguousarray(np.stack(
                [images[sl].reshape(P, FREE), noise[sl].reshape(P, FREE)],
                axis=1)),
            "meta": np.ascontiguousarray(meta),
        })
    return in_maps


def kernel(images, noise, center_h, center_w, half_h, half_w):
    global LAST_RESULTS
    from concourse.bass_utils import run_bass_kernel_spmd

    nc = _get_nc()
    in_maps = _make_in_maps(images, noise, center_h, center_w, half_h, half_w)
    trace = os.environ.get("KERNEL_TRACE", "0") == "1"
    if trace:
        from concourse._compat import axon_active
        if axon_active():
            try:
                import antenv.axon_hooks  # noqa: F401
            except ImportError:
                trace = False  # axon NTFF hook unavailable; run untraced
    res = run_bass_kernel_spmd(nc, in_maps, core_ids=list(range(M)),
                               trace=trace)
    LAST_RESULTS = res
    out = np.concatenate(
        [r["out"].reshape(PB, H, W, C) for r in res.results], axis=0)
    return out

